# revision 6
# baseline (speedup 1.0000x reference)
"""MoE gating kernel (logits -> softmax -> top-2 mask) for 8 trn2 NeuronCores.

Math: logits = x @ W.T + b  [B,S,E]; weights = softmax(logits, -1);
gated = weights masked to per-token top-2.  Returns (gated.T, weights.T),
both [E, B, S] fp32.

Strategy (v11, x-stationary):
  - Shard tokens (B*S = 65536) across 8 cores, 8192 tokens each.
  - fp32-class precision from fp16 splits with power-of-2 scales:
        x ~= A + 2^-11 * B                       (A, B fp16)
        logits*2^8 ~= A@C.T + A@Dp.T + B@Cs.T
    where C = fp16(W*2^8), Dp = fp16(W*2^8 - C), Cs = fp16(C*2^-11).
  - v10 streamed x as the MOVING operand (2 full 512-cycle passes per
    chunk-half); the PE was the bottleneck (~121us of matmul).  v11 makes
    x the STATIONARY operand instead: per (token-tile, 64-d-slice) one
    LDWEIGHTS of a 128x128 tile whose partitions 0:64 hold A rows and
    64:128 hold B rows (the two input DMAs land interleaved in one SBUF
    tile), then ONE F=48 matmul against the tiny packed W operand
        mv2[0:64 , kh, :] = [C | Dp | 0 ]   (d = kh*64 + p)
        mv2[64:128, kh, :] = [0 | 0  | Cs]
    accumulating [128 tok, 48] in PSUM over 16 kh-slices.  Moving-side
    work drops 8x; logits come out TOKEN-major so the per-tile logit
    transposes of v10 disappear.
  - Per 1024-token group: 2 DMAs (A/B halves, 2MB total, 2KB runs), 128
    LDW+MM pairs, then a batched tail: 2-strip combine -> logits*2^8
    [128, 8, 16], exp(scale 2^-8), segmented row-sum, reciprocal, max8
    per tile for the top-2 threshold, gate in two fused ops, and 2 PE
    transposes into the [(tile,e), (group,t)] output accumulators.
  - Outputs written once at the end with one strided DMA per output.
"""

import functools

import numpy as np

NUM_CORES = 8
TOK_PER_CORE = 8192
GROUPS = 8
GTOK = 1024
TILES = 8
KH = 16  # 64-row d-slices
D = 1024
E = 16

XS = 11  # x = A + 2^-XS * B
WS = 8  # accumulating logits * 2^WS

TRACE = False
LAST_RESULTS = None


@functools.lru_cache(maxsize=2)
def _build(has_b: bool):
    from concourse import bacc, mybir
    import concourse.bass as bass
    import concourse.tile as tile
    from concourse.masks import make_identity

    f16 = mybir.dt.float16
    f32 = mybir.dt.float32
    Exp = mybir.ActivationFunctionType.Exp
    Op = mybir.AluOpType
    X = mybir.AxisListType.X

    nc = bacc.Bacc(
        "TRN2", target_bir_lowering=False, debug=False, num_devices=NUM_CORES
    )

    # A.T / B.T shards: [1024 d, 8192 t] fp16, d-major
    at_dram = nc.dram_tensor("a_t", [D, TOK_PER_CORE], f16, kind="ExternalInput").ap()
    bt_dram = nc.dram_tensor("b_t", [D, TOK_PER_CORE], f16, kind="ExternalInput").ap()
    mv_dram = nc.dram_tensor("mv2", [128, KH, 3 * E], f16, kind="ExternalInput").ap()
    if has_b:
        bcd_dram = nc.dram_tensor("bcd", [2, 3 * E], f16, kind="ExternalInput").ap()
    wts_dram = nc.dram_tensor("wts", [E, TOK_PER_CORE], f32, kind="ExternalOutput")
    gated_dram = nc.dram_tensor("gated", [E, TOK_PER_CORE], f32, kind="ExternalOutput")

    def bcast_inner(ap, n):
        return bass.AP(tensor=ap.tensor, offset=ap.offset, ap=[*ap.ap, [0, n]])

    with tile.TileContext(nc) as tc:
        with (
            tc.tile_pool(name="consts", bufs=1) as consts,
            tc.tile_pool(name="xt", bufs=3) as xt_pool,
            tc.tile_pool(name="lg", bufs=2) as lg_pool,
            tc.tile_pool(name="sm", bufs=2) as sm_pool,
            tc.tile_pool(name="oacc", bufs=1) as oacc_pool,
            tc.tile_pool(name="pss", bufs=3, space="PSUM") as pss_pool,
            tc.tile_pool(name="psout", bufs=2, space="PSUM") as psout_pool,
        ):
            mv_sb = consts.tile([128, KH, 3 * E], f16)
            nc.sync.dma_start(out=mv_sb, in_=mv_dram)
            ident32 = consts.tile([128, 128], f32)
            make_identity(nc, ident32)
            if has_b:
                bcd_sb = consts.tile([2, 3 * E], f16)
                nc.sync.dma_start(out=bcd_sb, in_=bcd_dram)
                ones2 = consts.tile([2, 128], f16)
                nc.vector.memset(ones2[0:1, :], 1.0)
                nc.vector.memset(ones2[1:2, :], float(2.0**-XS))

            w_acc = oacc_pool.tile([128, GROUPS, 128], f32)
            g_acc = oacc_pool.tile([128, GROUPS, 128], f32)

            def mm_phase(g):
                xab = xt_pool.tile([128, KH, GTOK], f16, tag="xab")
                gs = slice(g * GTOK, (g + 1) * GTOK)
                nc.sync.dma_start(
                    out=xab[0:64, :, :],
                    in_=at_dram[:, gs].rearrange("(kh p) t -> p kh t", p=64),
                )
                nc.sync.dma_start(
                    out=xab[64:128, :, :],
                    in_=bt_dram[:, gs].rearrange("(kh p) t -> p kh t", p=64),
                )

                # flat PSUM tile (1536 B/partition = 1 bank); 3-D views of it
                # per-slice would make the allocator burn a bank per slice
                ps = pss_pool.tile([128, TILES * 3 * E], f32, tag="ps", name=f"ps_g{g}")
                psv = ps.rearrange("p (i c) -> p i c", c=3 * E)
                for i in range(TILES):
                    if has_b:
                        # bias via K=2 rank-1 matmul: row0 1.0 * bc, row1
                        # 2^-11 * bd; opens the accumulation over cols 0:48
                        nc.tensor.matmul(
                            psv[:, i, :], lhsT=ones2, rhs=bcd_sb,
                            start=True, stop=False,
                        )
                    for kh in range(KH):
                        nc.tensor.matmul(
                            psv[:, i, :],
                            lhsT=xab[:, kh, 128 * i : 128 * (i + 1)],
                            rhs=mv_sb[:, kh, :],
                            start=(kh == 0 and not has_b),
                            stop=(kh == KH - 1),
                        )
                return psv

            def tail_phase(g, ps):
                # logits*2^8 = strip0 + strip16 + strip32 (one PSUM input/op)
                cmb = sm_pool.tile([128, TILES, E], f32, tag="cmb")
                nc.scalar.copy(cmb, ps[:, :, 0:E])
                nc.vector.tensor_add(cmb, cmb, ps[:, :, E : 2 * E])
                lg = lg_pool.tile([128, TILES, E], f32, tag="lg", name=f"lg{g}")
                nc.vector.tensor_add(lg, cmb, ps[:, :, 2 * E : 3 * E])

                m8 = sm_pool.tile([128, TILES, 8], f32, tag="m8")
                for i in range(TILES):
                    nc.vector.max(m8[:, i, :], lg[:, i, :])
                ex = sm_pool.tile([128, TILES, E], f32, tag="ex")
                nc.scalar.activation(ex, lg, func=Exp, scale=float(2.0**-WS))
                ssum = sm_pool.tile([128, TILES], f32, tag="ssum")
                nc.vector.tensor_reduce(ssum, ex, axis=X, op=Op.add)
                rec = sm_pool.tile([128, TILES], f32, tag="rec")
                nc.vector.reciprocal(rec, ssum)
                w_grp = sm_pool.tile([128, TILES, E], f32, tag="wg")
                nc.vector.tensor_tensor(
                    out=w_grp, in0=ex, in1=bcast_inner(rec[:, :], E), op=Op.mult
                )
                msk = sm_pool.tile([128, TILES, E], f32, tag="msk")
                nc.vector.tensor_tensor(
                    out=msk, in0=lg, in1=bcast_inner(m8[:, :, 1], E), op=Op.is_ge
                )
                g_grp = sm_pool.tile([128, TILES, E], f32, tag="gg")
                nc.vector.tensor_tensor(out=g_grp, in0=msk, in1=w_grp, op=Op.mult)

                ps_o = psout_pool.tile([128, 256], f32)
                nc.tensor.transpose(ps_o[:, 0:128], w_grp, ident32)
                nc.tensor.transpose(ps_o[:, 128:256], g_grp, ident32)
                nc.scalar.copy(w_acc[:, g, :], ps_o[:, 0:128])
                nc.vector.tensor_copy(g_acc[:, g, :], ps_o[:, 128:256])

            # software pipeline: group g's matmuls, then group g-1's tail
            prev = None
            for g in range(GROUPS):
                ps = mm_phase(g)
                if prev is not None:
                    tail_phase(prev[0], prev[1])
                prev = (g, ps)
            tail_phase(prev[0], prev[1])

            # writeback: partition p=(tile,e); addr = e*8192 + g*1024 + tile*128 + t
            out_ap = [[128, TILES], [TOK_PER_CORE, E], [GTOK, GROUPS], [1, 128]]
            nc.sync.dma_start(
                out=bass.AP(tensor=wts_dram, offset=0, ap=list(out_ap)), in_=w_acc
            )
            nc.sync.dma_start(
                out=bass.AP(tensor=gated_dram, offset=0, ap=list(out_ap)), in_=g_acc
            )

    nc.compile()
    return nc


def _w_consts(W):
    C = (W * np.float32(2.0**WS)).astype(np.float16)
    Dp = (W * np.float32(2.0**WS) - C.astype(np.float32)).astype(np.float16)
    Cs = (C.astype(np.float32) * np.float32(2.0**-XS)).astype(np.float16)

    def lay64(M):  # [16, 1024] -> [64 p, KH, E] with d = kh*64 + p
        return np.ascontiguousarray(M.T.reshape(KH, 64, E).transpose(1, 0, 2))

    mv2 = np.zeros((128, KH, 3 * E), np.float16)
    mv2[0:64, :, 0:E] = lay64(C)
    mv2[0:64, :, E : 2 * E] = lay64(Dp)
    mv2[64:128, :, 2 * E : 3 * E] = lay64(Cs)
    return mv2


def kernel(x, W, b):
    global LAST_RESULTS
    from concourse.bass_utils import run_bass_kernel_spmd

    x = np.ascontiguousarray(np.asarray(x, dtype=np.float32))
    W = np.ascontiguousarray(np.asarray(W, dtype=np.float32))
    b = np.ascontiguousarray(np.asarray(b, dtype=np.float32))
    Bb, S, Dd = x.shape
    ntok = Bb * S
    assert (ntok, Dd) == (NUM_CORES * TOK_PER_CORE, D) and W.shape == (E, D)

    # fp16 hi/lo split, shipped d-major (transposed) per core
    xf = x.reshape(ntok, D)
    A = xf.astype(np.float16)
    Bx = ((xf - A.astype(np.float32)) * np.float32(2.0**XS)).astype(np.float16)
    AT = np.ascontiguousarray(A.T)  # [1024, 65536]
    BT = np.ascontiguousarray(Bx.T)

    mv2 = _w_consts(W)

    has_b = bool(np.any(b))
    in_maps = []
    for c in range(NUM_CORES):
        ts = slice(c * TOK_PER_CORE, (c + 1) * TOK_PER_CORE)
        m = {
            "a_t": np.ascontiguousarray(AT[:, ts]),
            "b_t": np.ascontiguousarray(BT[:, ts]),
            "mv2": mv2,
        }
        if has_b:
            bc = (b * np.float32(2.0**WS)).astype(np.float16)
            bd = ((b * np.float32(2.0**WS) - bc.astype(np.float32))
                  * np.float32(2.0**XS)).astype(np.float16)
            bcd = np.zeros((2, 3 * E), np.float16)
            bcd[0, 0:E] = bc
            bcd[1, 0:E] = bd
            m["bcd"] = bcd
        in_maps.append(m)

    nc = _build(has_b)
    res = run_bass_kernel_spmd(
        nc, in_maps, core_ids=list(range(NUM_CORES)), trace=TRACE
    )
    LAST_RESULTS = res

    wts = np.concatenate([r["wts"] for r in res.results], axis=1)
    gated = np.concatenate([r["gated"] for r in res.results], axis=1)
    return (
        gated.reshape(E, Bb, S).astype(np.float32),
        wts.reshape(E, Bb, S).astype(np.float32),
    )


# revision 10
# speedup vs baseline: 1.4147x; 1.4147x over previous
"""MoE gating kernel (logits -> softmax -> top-2 mask) for 8 trn2 NeuronCores.

Math: logits = x @ W.T + b  [B,S,E]; weights = softmax(logits, -1);
gated = weights masked to per-token top-2.  Returns (gated.T, weights.T),
both [E, B, S] fp32.

Strategy (v11, x-stationary):
  - Shard tokens (B*S = 65536) across 8 cores, 8192 tokens each.
  - fp32-class precision from fp16 splits with power-of-2 scales:
        x ~= A + 2^-11 * B                       (A, B fp16)
        logits*2^8 ~= A@C.T + A@Dp.T + B@Cs.T
    where C = fp16(W*2^8), Dp = fp16(W*2^8 - C), Cs = fp16(C*2^-11).
  - v10 streamed x as the MOVING operand (2 full 512-cycle passes per
    chunk-half); the PE was the bottleneck (~121us of matmul).  v11 makes
    x the STATIONARY operand instead: per (token-tile, 64-d-slice) one
    LDWEIGHTS of a 128x128 tile whose partitions 0:64 hold A rows and
    64:128 hold B rows (the two input DMAs land interleaved in one SBUF
    tile), then ONE F=48 matmul against the tiny packed W operand
        mv2[0:64 , kh, :] = [C | Dp | 0 ]   (d = kh*64 + p)
        mv2[64:128, kh, :] = [0 | 0  | Cs]
    accumulating [128 tok, 48] in PSUM over 16 kh-slices.  Moving-side
    work drops 8x; logits come out TOKEN-major so the per-tile logit
    transposes of v10 disappear.
  - Per 1024-token group: 2 DMAs (A/B halves, 2MB total, 2KB runs), 128
    LDW+MM pairs, then a batched tail: 2-strip combine -> logits*2^8
    [128, 8, 16], exp(scale 2^-8), segmented row-sum, reciprocal, max8
    per tile for the top-2 threshold, gate in two fused ops, and 2 PE
    transposes into the [(tile,e), (group,t)] output accumulators.
  - Outputs written once at the end with one strided DMA per output.
"""

import functools

import numpy as np

NUM_CORES = 8
TOK_PER_CORE = 8192
GROUPS = 8
GTOK = 1024
TILES = 8
KH = 16  # 64-row d-slices
D = 1024
E = 16

XS = 11  # x = A + 2^-XS * B
WS = 8  # accumulating logits * 2^WS

TRACE = False
LAST_RESULTS = None


@functools.lru_cache(maxsize=2)
def _build(has_b: bool):
    from concourse import bacc, mybir
    import concourse.bass as bass
    import concourse.tile as tile
    from concourse.masks import make_identity

    f16 = mybir.dt.float16
    f32 = mybir.dt.float32
    Exp = mybir.ActivationFunctionType.Exp
    Op = mybir.AluOpType
    X = mybir.AxisListType.X

    nc = bacc.Bacc(
        "TRN2", target_bir_lowering=False, debug=False, num_devices=NUM_CORES
    )

    # merged hi/lo shard: [2048, 8192] fp16, rows kh*128+p hold A-row kh*64+p
    # (p<64) / B-row kh*64+p-64 (p>=64) so ONE full-128-partition DMA per
    # group feeds all 16 SDMA engines (two 64-partition DMAs ran at half rate)
    xm_dram = nc.dram_tensor("xm", [2 * D, TOK_PER_CORE], f16, kind="ExternalInput").ap()
    mv_dram = nc.dram_tensor("mv2", [128, KH, 3 * E], f16, kind="ExternalInput").ap()
    if has_b:
        bcd_dram = nc.dram_tensor("bcd", [2, 3 * E], f16, kind="ExternalInput").ap()
    wts_dram = nc.dram_tensor("wts", [E, TOK_PER_CORE], f32, kind="ExternalOutput")
    gated_dram = nc.dram_tensor("gated", [E, TOK_PER_CORE], f32, kind="ExternalOutput")

    def bcast_inner(ap, n):
        return bass.AP(tensor=ap.tensor, offset=ap.offset, ap=[*ap.ap, [0, n]])

    with tile.TileContext(nc) as tc:
        with (
            tc.tile_pool(name="consts", bufs=1) as consts,
            tc.tile_pool(name="xt", bufs=3) as xt_pool,
            tc.tile_pool(name="lg", bufs=2) as lg_pool,
            tc.tile_pool(name="sm", bufs=2) as sm_pool,
            tc.tile_pool(name="oacc", bufs=1) as oacc_pool,
            tc.tile_pool(name="pss", bufs=3, space="PSUM") as pss_pool,
            tc.tile_pool(name="psout", bufs=2, space="PSUM") as psout_pool,
        ):
            mv_sb = consts.tile([128, KH, 3 * E], f16)
            nc.sync.dma_start(out=mv_sb, in_=mv_dram)
            ident32 = consts.tile([128, 128], f32)
            make_identity(nc, ident32)
            if has_b:
                bcd_sb = consts.tile([2, 3 * E], f16)
                nc.sync.dma_start(out=bcd_sb, in_=bcd_dram)
                ones2 = consts.tile([2, 128], f16)
                nc.vector.memset(ones2[0:1, :], 1.0)
                nc.vector.memset(ones2[1:2, :], float(2.0**-XS))

            w_acc = oacc_pool.tile([128, GROUPS, 128], f32)
            g_acc = oacc_pool.tile([128, GROUPS, 128], f32)

            def mm_phase(g):
                xab = xt_pool.tile([128, KH, GTOK], f16, tag="xab")
                gs = slice(g * GTOK, (g + 1) * GTOK)
                nc.sync.dma_start(
                    out=xab,
                    in_=xm_dram[:, gs].rearrange("(kh p) t -> p kh t", p=128),
                )

                # flat PSUM tile (1536 B/partition = 1 bank); 3-D views of it
                # per-slice would make the allocator burn a bank per slice
                ps = pss_pool.tile([128, TILES * 3 * E], f32, tag="ps", name=f"ps_g{g}")
                psv = ps.rearrange("p (i c) -> p i c", c=3 * E)
                for i in range(TILES):
                    if has_b:
                        # bias via K=2 rank-1 matmul: row0 1.0 * bc, row1
                        # 2^-11 * bd; opens the accumulation over cols 0:48
                        nc.tensor.matmul(
                            psv[:, i, :], lhsT=ones2, rhs=bcd_sb,
                            start=True, stop=False,
                        )
                    for kh in range(KH):
                        nc.tensor.matmul(
                            psv[:, i, :],
                            lhsT=xab[:, kh, 128 * i : 128 * (i + 1)],
                            rhs=mv_sb[:, kh, :],
                            start=(kh == 0 and not has_b),
                            stop=(kh == KH - 1),
                        )
                return psv

            def tail_phase(g, ps):
                # logits*2^8 = strip0 + strip16 + strip32 (one PSUM input/op)
                cmb = sm_pool.tile([128, TILES, E], f32, tag="cmb")
                nc.scalar.copy(cmb, ps[:, :, 0:E])
                nc.vector.tensor_add(cmb, cmb, ps[:, :, E : 2 * E])
                lg = lg_pool.tile([128, TILES, E], f32, tag="lg", name=f"lg{g}")
                nc.vector.tensor_add(lg, cmb, ps[:, :, 2 * E : 3 * E])

                m8 = sm_pool.tile([128, TILES, 8], f32, tag="m8")
                for i in range(TILES):
                    nc.vector.max(m8[:, i, :], lg[:, i, :])
                ex = sm_pool.tile([128, TILES, E], f32, tag="ex")
                nc.scalar.activation(ex, lg, func=Exp, scale=float(2.0**-WS))
                ssum = sm_pool.tile([128, TILES], f32, tag="ssum")
                nc.vector.tensor_reduce(ssum, ex, axis=X, op=Op.add)
                rec = sm_pool.tile([128, TILES], f32, tag="rec")
                nc.vector.reciprocal(rec, ssum)
                w_grp = sm_pool.tile([128, TILES, E], f32, tag="wg")
                nc.vector.tensor_tensor(
                    out=w_grp, in0=ex, in1=bcast_inner(rec[:, :], E), op=Op.mult
                )
                msk = sm_pool.tile([128, TILES, E], f32, tag="msk")
                nc.vector.tensor_tensor(
                    out=msk, in0=lg, in1=bcast_inner(m8[:, :, 1], E), op=Op.is_ge
                )
                g_grp = sm_pool.tile([128, TILES, E], f32, tag="gg")
                nc.vector.tensor_tensor(out=g_grp, in0=msk, in1=w_grp, op=Op.mult)

                ps_o = psout_pool.tile([128, 256], f32)
                nc.tensor.transpose(ps_o[:, 0:128], w_grp, ident32)
                nc.tensor.transpose(ps_o[:, 128:256], g_grp, ident32)
                nc.scalar.copy(w_acc[:, g, :], ps_o[:, 0:128])
                nc.vector.tensor_copy(g_acc[:, g, :], ps_o[:, 128:256])

            # software pipeline: group g's matmuls, then group g-1's tail
            prev = None
            for g in range(GROUPS):
                ps = mm_phase(g)
                if prev is not None:
                    tail_phase(prev[0], prev[1])
                prev = (g, ps)
            tail_phase(prev[0], prev[1])

            # writeback: partition p=(tile,e); addr = e*8192 + g*1024 + tile*128 + t
            out_ap = [[128, TILES], [TOK_PER_CORE, E], [GTOK, GROUPS], [1, 128]]
            nc.sync.dma_start(
                out=bass.AP(tensor=wts_dram, offset=0, ap=list(out_ap)), in_=w_acc
            )
            nc.sync.dma_start(
                out=bass.AP(tensor=gated_dram, offset=0, ap=list(out_ap)), in_=g_acc
            )

    nc.compile()
    return nc


def _w_consts(W):
    C = (W * np.float32(2.0**WS)).astype(np.float16)
    Dp = (W * np.float32(2.0**WS) - C.astype(np.float32)).astype(np.float16)
    Cs = (C.astype(np.float32) * np.float32(2.0**-XS)).astype(np.float16)

    def lay64(M):  # [16, 1024] -> [64 p, KH, E] with d = kh*64 + p
        return np.ascontiguousarray(M.T.reshape(KH, 64, E).transpose(1, 0, 2))

    mv2 = np.zeros((128, KH, 3 * E), np.float16)
    mv2[0:64, :, 0:E] = lay64(C)
    mv2[0:64, :, E : 2 * E] = lay64(Dp)
    mv2[64:128, :, 2 * E : 3 * E] = lay64(Cs)
    return mv2


def kernel(x, W, b):
    global LAST_RESULTS
    from concourse.bass_utils import run_bass_kernel_spmd

    x = np.ascontiguousarray(np.asarray(x, dtype=np.float32))
    W = np.ascontiguousarray(np.asarray(W, dtype=np.float32))
    b = np.ascontiguousarray(np.asarray(b, dtype=np.float32))
    Bb, S, Dd = x.shape
    ntok = Bb * S
    assert (ntok, Dd) == (NUM_CORES * TOK_PER_CORE, D) and W.shape == (E, D)

    # fp16 hi/lo split, shipped d-major (transposed) per core
    xf = x.reshape(ntok, D)
    A = xf.astype(np.float16)
    Bx = ((xf - A.astype(np.float32)) * np.float32(2.0**XS)).astype(np.float16)
    AT = A.T  # [1024, 65536]
    BT = Bx.T
    XM = np.empty((2 * D, ntok), np.float16)
    XMv = XM.reshape(KH, 128, ntok)
    XMv[:, 0:64] = AT.reshape(KH, 64, ntok)
    XMv[:, 64:128] = BT.reshape(KH, 64, ntok)

    mv2 = _w_consts(W)

    has_b = bool(np.any(b))
    in_maps = []
    for c in range(NUM_CORES):
        ts = slice(c * TOK_PER_CORE, (c + 1) * TOK_PER_CORE)
        m = {
            "xm": np.ascontiguousarray(XM[:, ts]),
            "mv2": mv2,
        }
        if has_b:
            bc = (b * np.float32(2.0**WS)).astype(np.float16)
            bd = ((b * np.float32(2.0**WS) - bc.astype(np.float32))
                  * np.float32(2.0**XS)).astype(np.float16)
            bcd = np.zeros((2, 3 * E), np.float16)
            bcd[0, 0:E] = bc
            bcd[1, 0:E] = bd
            m["bcd"] = bcd
        in_maps.append(m)

    nc = _build(has_b)
    res = run_bass_kernel_spmd(
        nc, in_maps, core_ids=list(range(NUM_CORES)), trace=TRACE
    )
    LAST_RESULTS = res

    wts = np.concatenate([r["wts"] for r in res.results], axis=1)
    gated = np.concatenate([r["gated"] for r in res.results], axis=1)
    return (
        gated.reshape(E, Bb, S).astype(np.float32),
        wts.reshape(E, Bb, S).astype(np.float32),
    )
